# revision 2
# baseline (speedup 1.0000x reference)
"""Trainium2 Bass kernel: NeRF fine-sampling (inverse-CDF sample + merge-sort).

Contract: kernel(**inputs) takes the FULL inputs
    dists         [262144, 64]  f32  (per-ray sorted distances)
    weights       [262144, 63]  f32
    uniform_rands [262144, 128] f32
    samples_fine  scalar (= 128)
and returns the FULL output [262144, 192] f32, equal to
    sort(concat(inverse_cdf_samples, dists), axis=-1).

Strategy: pure data-parallel over rays; 8 NeuronCores each take 32768 rays.

The execution backend charges a large, size-independent cost PER
INSTRUCTION, so the kernel is organized to minimize instruction count:
four passes over the rays, each at the widest ray-group G its SBUF
footprint allows, connected through Internal-DRAM scratch:

  pass0 (tables, G=64):  w1 = w+.01; per-window totals via masked scan +
         reversed ffill-max scan (broadcast without scatter); normalized
         pdf w1N, cdf cN = cumsum(w1N); slope sN = ddiff/w1N; d16/s16 =
         f16(dists), f16(slope) tables.
  pass1 (sort, G=128):   28-stage bitonic mergesort of each ray's 128 u's.
  pass2 (merge, G=64):   LSB-clear sorted u; build breakpoint run
         [-0.0|LSB, cN|LSB, +BIG pad]; 8-stage bitonic merge -> 192-long
         merged stream per ray (keys only; breakpoints tagged via LSB).
  pass3 (eval, G=32):    tag=LSB, C=cumsum(tag), ordinal=C*tag-1;
         posTab[ordinal]=position (local_scatter, chunk-local indices);
         scatter d16/s16 to breakpoint positions (16-bit scatters, chunk-
         aligned); masked-ffill scans propagate (d0, s0, cdf0);
         out = d0 + max(v-cdf0,0)*s0.  All interpolation identities are
         scale-consistent, and breakpoint entries evaluate to ~d_j exactly,
         reproducing the coarse dists in the merged result.
"""

from contextlib import ExitStack

import numpy as np

import concourse.bass as bass
import concourse.tile as tile
from concourse import bacc, mybir

F32 = mybir.dt.float32
F16 = mybir.dt.float16
I32 = mybir.dt.int32
I16 = mybir.dt.int16
Alu = mybir.AluOpType
Act = mybir.ActivationFunctionType

P = 128
SC = 64
NW = SC - 1      # 63
SF = 128
OUT = SF + SC    # 192
W = 256          # merge window

BIG = 1e30
DENORM_I = -2147483647      # 0x80000001 = -0.0|LSB  (~ -1.4e-45, tagged)

G0_MAX = 64      # tables pass
G1_MAX = 128     # sort pass
G2_MAX = 64      # merge pass
G3_MAX = 32      # eval pass

POS_CHUNK_ORD = 1024        # posTab scatter: ordinals per chunk (dst elems)
DS_CHUNK_POS = 1536         # d/s scatter: positions per chunk (dst elems)


def _r3(ap, inner):
    return ap.rearrange("p (g w) -> p g w", w=inner)


def emit_pass0(nc, tc, ctx, dists_ap, weights_ap, cN_ap, d16_ap, s16_ap,
               n_tiles):
    """Tables: normalized cdf cN, f16 dist/slope tables."""
    G = min(G0_MAX, n_tiles)
    assert n_tiles % G == 0
    NWG = G * NW
    pool = ctx.enter_context(tc.tile_pool(name="p0", bufs=1))

    # consts: wmask (0 at j%63==0 else 1), bias 0.01
    wmaski = pool.tile([P, NWG], I16)
    nc.gpsimd.iota(wmaski[:], pattern=[[0, G], [1, NW]], base=0,
                   channel_multiplier=0)
    wmask = pool.tile([P, NWG], F32)
    nc.scalar.activation(wmask[:], wmaski[:], Act.Copy)
    nc.vector.tensor_scalar(out=wmask[:], in0=wmask[:], scalar1=1.0,
                            scalar2=None, op0=Alu.min)
    bias01 = pool.tile([P, 1], F32)
    nc.vector.memset(bias01[:], 0.01)

    for t in range(0, n_tiles, G):
        r0, r1 = t * P * G // G, 0  # rays [t*P, (t+G)*P) in tile units
        rays0, rays1 = t * P, (t + G) * P
        dQ = pool.tile([P, G * SC], F32, tag="dQ")
        nc.sync.dma_start(
            dQ[:], dists_ap[rays0:rays1, :].rearrange("(p k) c -> p (k c)",
                                                      p=P))
        w1 = pool.tile([P, NWG], F32, tag="w1")
        nc.sync.dma_start(
            w1[:], weights_ap[rays0:rays1, :].rearrange("(p k) c -> p (k c)",
                                                        p=P))
        # w1 = w + 0.01 (in place)
        nc.scalar.activation(w1[:], w1[:], Act.Identity, bias=bias01[:])
        # raw cumsum per window
        craw = pool.tile([P, NWG], F32, tag="craw")
        nc.vector.tensor_tensor_scan(craw[:], wmask[:], w1[:], 0.0,
                                     Alu.mult, Alu.add)
        # window totals broadcast: reversed ffill-max scan
        scanB = pool.tile([P, NWG], F32, tag="scanB")
        nc.vector.tensor_tensor_scan(scanB[:], wmask[:], craw[:, ::-1], 0.0,
                                     Alu.mult, Alu.max)
        recB = pool.tile([P, NWG], F32, tag="recB")
        nc.vector.reciprocal(recB[:], scanB[:])
        # normalized pdf (in place over w1)
        nc.vector.tensor_tensor(w1[:], w1[:], recB[:, ::-1], Alu.mult)
        # normalized cdf
        cN = pool.tile([P, NWG], F32, tag="cN")
        nc.vector.tensor_tensor_scan(cN[:], wmask[:], w1[:], 0.0,
                                     Alu.mult, Alu.add)
        # slope = ddiff / w1N
        dQ3 = _r3(dQ[:], SC)
        ddiff = pool.tile([P, NWG], F32, tag="ddiff")
        ddiff3 = _r3(ddiff[:], NW)
        nc.vector.tensor_tensor(ddiff3[:, :, :], dQ3[:, :, 1:SC],
                                dQ3[:, :, 0:NW], Alu.subtract)
        nc.vector.reciprocal(w1[:], w1[:])          # in-place recip
        nc.vector.tensor_tensor(ddiff[:], ddiff[:], w1[:], Alu.mult)
        # f16 tables
        d16 = pool.tile([P, G * SC], F16, tag="d16")
        nc.scalar.activation(d16[:], dQ[:], Act.Copy)
        s16 = pool.tile([P, G * SC], F16, tag="s16")
        nc.vector.memset(s16[:], 0.0)
        nc.scalar.activation(_r3(s16[:], SC)[:, :, 0:NW], ddiff[:], Act.Copy)
        # store
        nc.sync.dma_start(
            cN_ap[rays0:rays1, :].rearrange("(p k) c -> p (k c)", p=P), cN[:])
        nc.sync.dma_start(
            d16_ap[rays0:rays1, :].rearrange("(p k) c -> p (k c)", p=P),
            d16[:])
        nc.sync.dma_start(
            s16_ap[rays0:rays1, :].rearrange("(p k) c -> p (k c)", p=P),
            s16[:])


def emit_pass1(nc, tc, ctx, u_ap, usort_ap, n_tiles):
    """28-stage bitonic mergesort of u ascending, per ray."""
    G = min(G1_MAX, n_tiles)
    assert n_tiles % G == 0
    pool = ctx.enter_context(tc.tile_pool(name="p1", bufs=1))
    for t in range(0, n_tiles, G):
        rays0, rays1 = t * P, (t + G) * P
        V = pool.tile([P, G * SF], F32, tag="V")
        nc.sync.dma_start(
            V[:], u_ap[rays0:rays1, :].rearrange("(p k) c -> p (k c)", p=P))
        V2 = pool.tile([P, G * SF], F32, tag="V2")
        bufs = [V, V2]
        cur = 0
        for lev in range(1, 8):          # k = 2, 4, ..., 128
            k = 1 << lev
            h = k // 2
            src = bufs[cur][:].rearrange("p (c b) -> p c b", b=k)
            dst = bufs[1 - cur][:].rearrange("p (c b) -> p c b", b=k)
            lo_rev = src[:, :, h - 1::-1]
            hi = src[:, :, h:k]
            nc.vector.tensor_tensor(dst[:, :, 0:h], lo_rev, hi, Alu.min)
            nc.vector.tensor_tensor(dst[:, :, h:k], lo_rev, hi, Alu.max)
            cur = 1 - cur
            s = k // 4
            while s >= 1:
                src = bufs[cur][:].rearrange("p (c b) -> p c b", b=2 * s)
                dst = bufs[1 - cur][:].rearrange("p (c b) -> p c b", b=2 * s)
                nc.vector.tensor_tensor(dst[:, :, 0:s], src[:, :, 0:s],
                                        src[:, :, s:2 * s], Alu.min)
                nc.vector.tensor_tensor(dst[:, :, s:2 * s], src[:, :, 0:s],
                                        src[:, :, s:2 * s], Alu.max)
                cur = 1 - cur
                s //= 2
        assert cur == 0
        nc.sync.dma_start(
            usort_ap[rays0:rays1, :].rearrange("(p k) c -> p (k c)", p=P),
            V[:])


def emit_pass2(nc, tc, ctx, usort_ap, cN_ap, merged_ap, n_tiles):
    """Bitonic merge of sorted u with tagged breakpoint run."""
    G = min(G2_MAX, n_tiles)
    assert n_tiles % G == 0
    pool = ctx.enter_context(tc.tile_pool(name="p2", bufs=1))
    for t in range(0, n_tiles, G):
        rays0, rays1 = t * P, (t + G) * P
        Vs = pool.tile([P, G * SF], F32, tag="Vs")
        nc.sync.dma_start(
            Vs[:], usort_ap[rays0:rays1, :].rearrange("(p k) c -> p (k c)",
                                                      p=P))
        cN = pool.tile([P, G * NW], F32, tag="cN")
        nc.sync.dma_start(
            cN[:], cN_ap[rays0:rays1, :].rearrange("(p k) c -> p (k c)", p=P))
        # clear LSB of u (tag bit 0)
        nc.vector.tensor_scalar(out=Vs[:].bitcast(I32), in0=Vs[:].bitcast(I32),
                                scalar1=-2, scalar2=None, op0=Alu.bitwise_and)
        XT = pool.tile([P, G * W], F32, tag="XT")
        XT3 = _r3(XT[:], W)
        XT3_i = _r3(XT[:].bitcast(I32), W)
        nc.vector.memset(XT[:], BIG)
        nc.vector.memset(XT3_i[:, :, 128:129], DENORM_I)
        # cN with LSB set -> XT[129:192]
        nc.vector.tensor_scalar(out=XT3_i[:, :, 129:192],
                                in0=_r3(cN[:], NW).bitcast(I32),
                                scalar1=1, scalar2=None, op0=Alu.bitwise_or)
        # merge: stage 1 reads u reversed + bp run, then 7 cleaner stages
        Y = pool.tile([P, G * W], F32, tag="Y")
        Y3 = _r3(Y[:], W)
        Vs3 = _r3(Vs[:], SF)
        nc.vector.tensor_tensor(Y3[:, :, 0:128], Vs3[:, :, SF - 1::-1],
                                XT3[:, :, 128:256], Alu.min)
        nc.vector.tensor_tensor(Y3[:, :, 128:256], Vs3[:, :, SF - 1::-1],
                                XT3[:, :, 128:256], Alu.max)
        bufs = [Y, XT]
        s = W // 4
        idx = 0
        while s >= 1:
            src = bufs[idx % 2][:].rearrange("p (a b) -> p a b", b=2 * s)
            dst = bufs[(idx + 1) % 2][:].rearrange("p (a b) -> p a b", b=2 * s)
            nc.vector.tensor_tensor(dst[:, :, 0:s], src[:, :, 0:s],
                                    src[:, :, s:2 * s], Alu.min)
            nc.vector.tensor_tensor(dst[:, :, s:2 * s], src[:, :, 0:s],
                                    src[:, :, s:2 * s], Alu.max)
            s //= 2
            idx += 1
        assert idx % 2 == 1              # result back in XT
        nc.sync.dma_start(
            merged_ap[rays0:rays1, :].rearrange("(p k) c -> p k c", p=P),
            XT3[:, :, 0:OUT])


def emit_pass3(nc, tc, ctx, merged_ap, d16_ap, s16_ap, out_ap, n_tiles):
    """Tag, position-scatter, ffill, interpolate."""
    G = min(G3_MAX, n_tiles)
    assert n_tiles % G == 0
    SG = G * OUT                      # stream length (6144 at G=32)
    ORD = G * SC                      # ordinals (2048 at G=32)
    n_pos_chunks = (ORD + POS_CHUNK_ORD - 1) // POS_CHUNK_ORD
    n_ds_chunks = (SG + DS_CHUNK_POS - 1) // DS_CHUNK_POS
    assert ORD % n_pos_chunks == 0 and SG % n_ds_chunks == 0
    assert ORD // n_pos_chunks <= 2046 and SG // n_ds_chunks <= 2046
    pos_ord = ORD // n_pos_chunks     # ordinals per pos chunk
    pos_pos = SG // n_pos_chunks      # positions per pos chunk
    ds_pos = SG // n_ds_chunks        # positions per ds chunk
    ds_ord = ORD // n_ds_chunks       # ordinals per ds chunk
    pool = ctx.enter_context(tc.tile_pool(name="p3", bufs=1))

    # const: chunk-local position values for posTab scatter data
    iotaPos = pool.tile([P, SG], I16)
    nc.gpsimd.iota(iotaPos[:], pattern=[[0, n_ds_chunks], [1, ds_pos]],
                   base=0, channel_multiplier=0)

    for t in range(0, n_tiles, G):
        rays0, rays1 = t * P, (t + G) * P
        M = pool.tile([P, SG], F32, tag="M")
        nc.sync.dma_start(
            M[:], merged_ap[rays0:rays1, :].rearrange("(p k) c -> p (k c)",
                                                      p=P))
        d16T = pool.tile([P, ORD], F16, tag="d16T")
        nc.sync.dma_start(
            d16T[:], d16_ap[rays0:rays1, :].rearrange("(p k) c -> p (k c)",
                                                      p=P))
        s16T = pool.tile([P, ORD], F16, tag="s16T")
        nc.sync.dma_start(
            s16T[:], s16_ap[rays0:rays1, :].rearrange("(p k) c -> p (k c)",
                                                      p=P))
        # tag = v & 1 as f32 (in-place int->float via bitcast view)
        tagF = pool.tile([P, SG], F32, tag="tagF")
        nc.vector.tensor_scalar(out=tagF[:].bitcast(I32), in0=M[:].bitcast(I32),
                                scalar1=1, scalar2=None, op0=Alu.bitwise_and)
        nc.scalar.activation(tagF[:], tagF[:].bitcast(I32), Act.Copy)
        omt = pool.tile([P, SG], F32, tag="omt")
        nc.vector.tensor_scalar(out=omt[:], in0=tagF[:], scalar1=-1.0,
                                scalar2=1.0, op0=Alu.mult, op1=Alu.add)
        # C = cumsum(tag); ordinal = C*tag - 1 (chunk-rebased)
        C = pool.tile([P, SG], F32, tag="C")
        nc.vector.tensor_tensor_scan(C[:], tagF[:], tagF[:], 0.0,
                                     Alu.add, Alu.bypass)
        nc.vector.tensor_tensor(C[:], C[:], tagF[:], Alu.mult)
        for c in range(n_pos_chunks):
            nc.vector.tensor_scalar(
                out=C[:, c * pos_pos:(c + 1) * pos_pos],
                in0=C[:, c * pos_pos:(c + 1) * pos_pos],
                scalar1=float(-1 - c * pos_ord), scalar2=None, op0=Alu.add)
        idx16 = pool.tile([P, SG], I16, tag="idx16")
        nc.scalar.activation(idx16[:], C[:], Act.Copy)
        # posTab[ordinal] = chunk-local position
        posTab = pool.tile([P, ORD], I16, tag="posTab")
        for c in range(n_pos_chunks):
            nc.gpsimd.local_scatter(
                posTab[:, c * pos_ord:(c + 1) * pos_ord],
                iotaPos[:, c * pos_pos:(c + 1) * pos_pos],
                idx16[:, c * pos_pos:(c + 1) * pos_pos],
                channels=P, num_elems=pos_ord, num_idxs=pos_pos)
        # scatter d16/s16 to breakpoint positions
        dS = pool.tile([P, SG], F16, tag="dS")
        sS = pool.tile([P, SG], F16, tag="sS")
        for c in range(n_ds_chunks):
            osl = slice(c * ds_ord, (c + 1) * ds_ord)
            psl = slice(c * ds_pos, (c + 1) * ds_pos)
            nc.gpsimd.local_scatter(dS[:, psl], d16T[:, osl], posTab[:, osl],
                                    channels=P, num_elems=ds_pos,
                                    num_idxs=ds_ord)
            nc.gpsimd.local_scatter(sS[:, psl], s16T[:, osl], posTab[:, osl],
                                    channels=P, num_elems=ds_pos,
                                    num_idxs=ds_ord)
        # ffills (in place)
        nc.vector.tensor_tensor_scan(dS[:], omt[:], dS[:], 0.0,
                                     Alu.mult, Alu.add)
        nc.vector.tensor_tensor_scan(sS[:], omt[:], sS[:], 0.0,
                                     Alu.mult, Alu.add)
        # cdf0 ffill: cdfAt = v*tag (denorm is -0.0-ish -> ~0)
        cdfAt = pool.tile([P, SG], F32, tag="cdfAt")
        nc.vector.tensor_tensor(cdfAt[:], M[:], tagF[:], Alu.mult)
        nc.vector.tensor_tensor_scan(cdfAt[:], omt[:], cdfAt[:], 0.0,
                                     Alu.mult, Alu.add)
        # out = d0 + max(v - cdf0, 0) * s0
        nc.vector.tensor_tensor(tagF[:], M[:], cdfAt[:], Alu.subtract)
        s0 = pool.tile([P, SG], F32, tag="s0")
        nc.scalar.activation(s0[:], sS[:], Act.Copy)
        nc.vector.scalar_tensor_tensor(s0[:], tagF[:], 0.0, s0[:],
                                       Alu.max, Alu.mult)
        nc.scalar.activation(M[:], dS[:], Act.Copy)   # d0 into M's buffer
        nc.vector.tensor_tensor(s0[:], s0[:], M[:], Alu.add)
        nc.sync.dma_start(
            out_ap[rays0:rays1, :].rearrange("(p k) c -> p (k c)", p=P),
            s0[:])


def build_body(tc, ctx, nc, dists_ap, weights_ap, u_ap, out_ap, n_tiles,
               repeat=1):
    dram = ctx.enter_context(
        tc.tile_pool(name="scratch", bufs=1, space="DRAM"))
    n_rays = n_tiles * P
    cN_t = dram.tile([n_rays, NW], F32)
    d16_t = dram.tile([n_rays, SC], F16)
    s16_t = dram.tile([n_rays, SC], F16)
    usort_t = dram.tile([n_rays, SF], F32)
    merged_t = dram.tile([n_rays, OUT], F32)

    for _ in range(repeat):
        with ExitStack() as pctx:
            emit_pass0(nc, tc, pctx, dists_ap, weights_ap, cN_t[:], d16_t[:],
                       s16_t[:], n_tiles)
        with ExitStack() as pctx:
            emit_pass1(nc, tc, pctx, u_ap, usort_t[:], n_tiles)
        with ExitStack() as pctx:
            emit_pass2(nc, tc, pctx, usort_t[:], cN_t[:], merged_t[:], n_tiles)
        with ExitStack() as pctx:
            emit_pass3(nc, tc, pctx, merged_t[:], d16_t[:], s16_t[:], out_ap,
                       n_tiles)


def build_kernel(n_rays, repeat=1):
    assert n_rays % (P * G3_MAX) == 0
    nc = bacc.Bacc("TRN2", target_bir_lowering=False, debug=False)
    dists = nc.dram_tensor("dists", [n_rays, SC], F32,
                           kind="ExternalInput").ap()
    weights = nc.dram_tensor("weights", [n_rays, NW], F32,
                             kind="ExternalInput").ap()
    u = nc.dram_tensor("u", [n_rays, SF], F32, kind="ExternalInput").ap()
    out = nc.dram_tensor("out", [n_rays, OUT], F32,
                         kind="ExternalOutput").ap()
    with tile.TileContext(nc) as tc:
        with ExitStack() as ctx:
            build_body(tc, ctx, nc, dists, weights, u, out, n_rays // P,
                       repeat=repeat)
    nc.compile()
    return nc


N_CORES = 8
B_FULL = 262144
R_CORE = B_FULL // N_CORES   # 32768 rays per core

_NC_CACHE = {}


def _get_nc(n_rays, repeat=1):
    key = (n_rays, repeat)
    if key not in _NC_CACHE:
        _NC_CACHE[key] = build_kernel(n_rays, repeat)
    return _NC_CACHE[key]


def kernel(dists, weights, uniform_rands, samples_fine):
    from concourse.bass_utils import run_bass_kernel_spmd
    dists = np.ascontiguousarray(np.asarray(dists, dtype=np.float32))
    weights = np.ascontiguousarray(np.asarray(weights, dtype=np.float32))
    u = np.ascontiguousarray(np.asarray(uniform_rands, dtype=np.float32))
    assert int(samples_fine) == SF
    B = dists.shape[0]
    assert B == B_FULL and dists.shape[1] == SC and weights.shape[1] == NW \
        and u.shape[1] == SF

    nc = _get_nc(R_CORE)
    in_maps = []
    for c in range(N_CORES):
        r0, r1 = c * R_CORE, (c + 1) * R_CORE
        in_maps.append({"dists": dists[r0:r1], "weights": weights[r0:r1],
                        "u": u[r0:r1]})
    res = run_bass_kernel_spmd(nc, in_maps, list(range(N_CORES)))
    return np.concatenate([res.results[c]["out"] for c in range(N_CORES)],
                          axis=0)


# revision 9
# speedup vs baseline: 3.3551x; 3.3551x over previous
"""Trainium2 Bass kernel: NeRF fine-sampling (inverse-CDF sample + merge-sort).

Contract: kernel(**inputs) takes the FULL inputs
    dists         [262144, 64]  f32  (per-ray sorted distances)
    weights       [262144, 63]  f32
    uniform_rands [262144, 128] f32
    samples_fine  scalar (= 128)
and returns the FULL output [262144, 192] f32, equal to
    sort(concat(inverse_cdf_samples, dists), axis=-1).

Strategy: pure data-parallel over rays; 8 NeuronCores each take 32768 rays.

The execution backend charges a large, size-independent cost PER
INSTRUCTION, so the kernel minimizes instruction count: four passes over
the rays, each at the widest ray-group G its SBUF footprint allows,
connected through Internal-DRAM scratch:

  pass0 (tables, G=64):  w1 = w+.01; raw cdf craw = masked-scan cumsum;
         per-window reciprocal broadcast via stride-0 APs; normalized
         cdf cN and slope sN = ddiff/(w1*rec); f16 dist/slope tables.
  pass1 (sort, G=128):   28-stage bitonic mergesort of each ray's 128 u's.
  pass2 (merge, G=64):   LSB-clear sorted u; breakpoint run
         [-0.0|LSB, cN|LSB, +BIG pad]; 8-stage bitonic merge -> 192
         merged keys per ray (breakpoints tagged via LSB).
  pass3 (eval, G=32):    tag=LSB, C=cumsum(tag), ordinal=C*tag-1;
         posTab[ordinal]=position; scatter f16 d/slope tables to
         breakpoint positions (16-bit local_scatters, chunk-aligned
         local indices); masked-ffill scans propagate (d0, s0, cdf0);
         out = d0 + max(v-cdf0,0)*s0.  Breakpoint entries evaluate to
         ~d_j exactly, reproducing the coarse dists in the merged result.
"""

from contextlib import ExitStack

import numpy as np

import concourse.bass as bass
import concourse.tile as tile
from concourse import bacc, mybir

F32 = mybir.dt.float32
F16 = mybir.dt.float16
I32 = mybir.dt.int32
I16 = mybir.dt.int16
Alu = mybir.AluOpType
Act = mybir.ActivationFunctionType

P = 128
SC = 64
NW = SC - 1      # 63
SF = 128
OUT = SF + SC    # 192
W = 256          # merge window

BIG = 1e30
DENORM_I = -2147483647      # 0x80000001 = -0.0|LSB  (~ -1.4e-45, tagged)

G0_MAX = 64      # tables pass
G1_MAX = 128     # sort pass
G2_MAX = 64      # merge pass
G3_MAX = 32      # eval pass

POS_CHUNK_ORD = 1024        # posTab scatter: ordinals per chunk (dst elems)
DS_CHUNK_POS = 1536         # d/s scatter: positions per chunk (dst elems)


def _r3(ap, inner):
    return ap.rearrange("p (g w) -> p g w", w=inner)


def emit_pass0(nc, tc, ctx, dists_ap, weights_ap, cN_ap, d16_ap, s16_ap,
               n_tiles):
    """Tables: normalized cdf cN, f16 dist/slope tables."""
    G = min(G0_MAX, n_tiles)
    assert n_tiles % G == 0
    NWG = G * NW
    pool = ctx.enter_context(tc.tile_pool(name="p0", bufs=1))

    # const: wmask (0 at j%63==0 else 1)
    wmaski = pool.tile([P, NWG], I16)
    nc.gpsimd.iota(wmaski[:], pattern=[[0, G], [1, NW]], base=0,
                   channel_multiplier=0)
    wmask = pool.tile([P, NWG], F32)
    nc.vector.tensor_scalar(out=wmask[:], in0=wmaski[:], scalar1=1.0,
                            scalar2=None, op0=Alu.min)

    for t in range(0, n_tiles, G):
        rays0, rays1 = t * P, (t + G) * P
        dQ = pool.tile([P, G * SC], F32, tag="dQ")
        nc.sync.dma_start(
            dQ[:], dists_ap[rays0:rays1, :].rearrange("(p k) c -> p (k c)",
                                                      p=P))
        w1 = pool.tile([P, NWG], F32, tag="w1")
        nc.sync.dma_start(
            w1[:], weights_ap[rays0:rays1, :].rearrange("(p k) c -> p (k c)",
                                                        p=P))
        # w1 = w + 0.01 (in place)
        nc.vector.tensor_scalar(out=w1[:], in0=w1[:], scalar1=0.01,
                                scalar2=None, op0=Alu.add)
        # raw cumsum per window
        craw = pool.tile([P, NWG], F32, tag="craw")
        nc.vector.tensor_tensor_scan(craw[:], wmask[:], w1[:], 0.0,
                                     Alu.mult, Alu.add)
        # per-window reciprocal of total, broadcast via stride-0 AP
        recC = pool.tile([P, G], F32, tag="recC")
        nc.vector.reciprocal(recC[:], craw[:, NW - 1::NW])
        recB = recC[:].rearrange("p (g w) -> p g w", w=1).broadcast_to(
            (P, G, NW))
        # normalized cdf and pdf
        cN = pool.tile([P, NWG], F32, tag="cN")
        nc.vector.tensor_tensor(_r3(cN[:], NW), _r3(craw[:], NW), recB,
                                Alu.mult)
        nc.vector.tensor_tensor(_r3(w1[:], NW), _r3(w1[:], NW), recB,
                                Alu.mult)
        # slope = ddiff / w1N
        dQ3 = _r3(dQ[:], SC)
        ddiff = pool.tile([P, NWG], F32, tag="ddiff")
        nc.vector.tensor_tensor(_r3(ddiff[:], NW), dQ3[:, :, 1:SC],
                                dQ3[:, :, 0:NW], Alu.subtract)
        nc.vector.reciprocal(w1[:], w1[:])          # in-place recip
        nc.vector.tensor_tensor(ddiff[:], ddiff[:], w1[:], Alu.mult)
        # f16 tables
        d16 = pool.tile([P, G * SC], F16, tag="d16")
        nc.vector.tensor_copy(d16[:], dQ[:])
        s16 = pool.tile([P, G * SC], F16, tag="s16")
        nc.vector.memset(s16[:], 0.0)
        nc.vector.tensor_copy(_r3(s16[:], SC)[:, :, 0:NW], _r3(ddiff[:], NW))
        # store
        nc.sync.dma_start(
            cN_ap[rays0:rays1, :].rearrange("(p k) c -> p (k c)", p=P), cN[:])
        nc.sync.dma_start(
            d16_ap[rays0:rays1, :].rearrange("(p k) c -> p (k c)", p=P),
            d16[:])
        nc.sync.dma_start(
            s16_ap[rays0:rays1, :].rearrange("(p k) c -> p (k c)", p=P),
            s16[:])


def emit_pass1(nc, tc, ctx, u_ap, usort_ap, n_tiles):
    """28-stage bitonic mergesort of u ascending, per ray."""
    G = min(G1_MAX, n_tiles)
    assert n_tiles % G == 0
    pool = ctx.enter_context(tc.tile_pool(name="p1", bufs=1))
    for t in range(0, n_tiles, G):
        rays0, rays1 = t * P, (t + G) * P
        V = pool.tile([P, G * SF], F32, tag="V")
        nc.sync.dma_start(
            V[:], u_ap[rays0:rays1, :].rearrange("(p k) c -> p (k c)", p=P))
        V2 = pool.tile([P, G * SF], F32, tag="V2")
        bufs = [V, V2]
        cur = 0
        for lev in range(1, 8):          # k = 2, 4, ..., 128
            k = 1 << lev
            h = k // 2
            src = bufs[cur][:].rearrange("p (c b) -> p c b", b=k)
            dst = bufs[1 - cur][:].rearrange("p (c b) -> p c b", b=k)
            lo_rev = src[:, :, h - 1::-1]
            hi = src[:, :, h:k]
            nc.vector.tensor_tensor(dst[:, :, 0:h], lo_rev, hi, Alu.min)
            nc.vector.tensor_tensor(dst[:, :, h:k], lo_rev, hi, Alu.max)
            cur = 1 - cur
            s = k // 4
            while s >= 1:
                src = bufs[cur][:].rearrange("p (c b) -> p c b", b=2 * s)
                dst = bufs[1 - cur][:].rearrange("p (c b) -> p c b", b=2 * s)
                nc.vector.tensor_tensor(dst[:, :, 0:s], src[:, :, 0:s],
                                        src[:, :, s:2 * s], Alu.min)
                nc.vector.tensor_tensor(dst[:, :, s:2 * s], src[:, :, 0:s],
                                        src[:, :, s:2 * s], Alu.max)
                cur = 1 - cur
                s //= 2
        assert cur == 0
        nc.sync.dma_start(
            usort_ap[rays0:rays1, :].rearrange("(p k) c -> p (k c)", p=P),
            V[:])


def emit_pass2(nc, tc, ctx, usort_ap, cN_ap, merged_ap, n_tiles):
    """Bitonic merge of sorted u with tagged breakpoint run."""
    G = min(G2_MAX, n_tiles)
    assert n_tiles % G == 0
    pool = ctx.enter_context(tc.tile_pool(name="p2", bufs=1))
    for t in range(0, n_tiles, G):
        rays0, rays1 = t * P, (t + G) * P
        Vs = pool.tile([P, G * SF], F32, tag="Vs")
        nc.sync.dma_start(
            Vs[:], usort_ap[rays0:rays1, :].rearrange("(p k) c -> p (k c)",
                                                      p=P))
        cN = pool.tile([P, G * NW], F32, tag="cN")
        nc.sync.dma_start(
            cN[:], cN_ap[rays0:rays1, :].rearrange("(p k) c -> p (k c)", p=P))
        # clear LSB of u (tag bit 0)
        nc.vector.tensor_scalar(out=Vs[:].bitcast(I32), in0=Vs[:].bitcast(I32),
                                scalar1=-2, scalar2=None, op0=Alu.bitwise_and)
        XT = pool.tile([P, G * W], F32, tag="XT")
        XT3 = _r3(XT[:], W)
        XT3_i = _r3(XT[:].bitcast(I32), W)
        nc.vector.memset(XT[:], BIG)
        nc.vector.memset(XT3_i[:, :, 128:129], DENORM_I)
        # cN with LSB set -> XT[129:192]
        nc.vector.tensor_scalar(out=XT3_i[:, :, 129:192],
                                in0=_r3(cN[:], NW).bitcast(I32),
                                scalar1=1, scalar2=None, op0=Alu.bitwise_or)
        # merge: stage 1 reads u reversed + bp run, then 7 cleaner stages
        Y = pool.tile([P, G * W], F32, tag="Y")
        Y3 = _r3(Y[:], W)
        Vs3 = _r3(Vs[:], SF)
        nc.vector.tensor_tensor(Y3[:, :, 0:128], Vs3[:, :, SF - 1::-1],
                                XT3[:, :, 128:256], Alu.min)
        nc.vector.tensor_tensor(Y3[:, :, 128:256], Vs3[:, :, SF - 1::-1],
                                XT3[:, :, 128:256], Alu.max)
        bufs = [Y, XT]
        s = W // 4
        idx = 0
        while s >= 1:
            src = bufs[idx % 2][:].rearrange("p (a b) -> p a b", b=2 * s)
            dst = bufs[(idx + 1) % 2][:].rearrange("p (a b) -> p a b", b=2 * s)
            nc.vector.tensor_tensor(dst[:, :, 0:s], src[:, :, 0:s],
                                    src[:, :, s:2 * s], Alu.min)
            nc.vector.tensor_tensor(dst[:, :, s:2 * s], src[:, :, 0:s],
                                    src[:, :, s:2 * s], Alu.max)
            s //= 2
            idx += 1
        assert idx % 2 == 1              # result back in XT
        # store full 256-wide windows (contiguous DMA; pads unused later)
        nc.sync.dma_start(
            merged_ap[rays0:rays1, :].rearrange("(p k) c -> p (k c)", p=P),
            XT[:])


def emit_pass3(nc, tc, ctx, merged_ap, d16_ap, s16_ap, out_ap, n_tiles):
    """Tag, position-scatter, ffill, interpolate."""
    G = min(G3_MAX, n_tiles)
    assert n_tiles % G == 0
    SG = G * OUT                      # stream length (6144 at G=32)
    ORD = G * SC                      # ordinals (2048 at G=32)
    n_pos_chunks = (ORD + POS_CHUNK_ORD - 1) // POS_CHUNK_ORD
    n_ds_chunks = (SG + DS_CHUNK_POS - 1) // DS_CHUNK_POS
    assert ORD % n_pos_chunks == 0 and SG % n_ds_chunks == 0
    assert ORD // n_pos_chunks <= 2046 and SG // n_ds_chunks <= 2046
    pos_ord = ORD // n_pos_chunks     # ordinals per pos chunk
    pos_pos = SG // n_pos_chunks      # positions per pos chunk
    ds_pos = SG // n_ds_chunks        # positions per ds chunk
    ds_ord = ORD // n_ds_chunks       # ordinals per ds chunk
    pool = ctx.enter_context(tc.tile_pool(name="p3", bufs=1))

    # const: chunk-local position values for posTab scatter data
    iotaPos = pool.tile([P, SG], I16)
    nc.gpsimd.iota(iotaPos[:], pattern=[[0, n_ds_chunks], [1, ds_pos]],
                   base=0, channel_multiplier=0)

    for t in range(0, n_tiles, G):
        rays0, rays1 = t * P, (t + G) * P
        M = pool.tile([P, G * W], F32, tag="M")
        nc.sync.dma_start(
            M[:], merged_ap[rays0:rays1, :].rearrange("(p k) c -> p (k c)",
                                                      p=P))
        Mv = _r3(M[:], W)[:, :, 0:OUT]              # [P, G, 192] strided
        Mvi = _r3(M[:].bitcast(I32), W)[:, :, 0:OUT]
        d16T = pool.tile([P, ORD], F16, tag="d16T")
        nc.sync.dma_start(
            d16T[:], d16_ap[rays0:rays1, :].rearrange("(p k) c -> p (k c)",
                                                      p=P))
        s16T = pool.tile([P, ORD], F16, tag="s16T")
        nc.sync.dma_start(
            s16T[:], s16_ap[rays0:rays1, :].rearrange("(p k) c -> p (k c)",
                                                      p=P))
        # tag = v & 1 as f32 (int->float conversion on vector engine)
        tagF = pool.tile([P, SG], F32, tag="tagF")
        nc.vector.tensor_scalar(out=_r3(tagF[:].bitcast(I32), OUT), in0=Mvi,
                                scalar1=1, scalar2=None, op0=Alu.bitwise_and)
        nc.vector.tensor_scalar(out=tagF[:], in0=tagF[:].bitcast(I32),
                                scalar1=1.0, scalar2=None, op0=Alu.mult)
        omt = pool.tile([P, SG], F32, tag="omt")
        nc.vector.tensor_scalar(out=omt[:], in0=tagF[:], scalar1=-1.0,
                                scalar2=1.0, op0=Alu.mult, op1=Alu.add)
        # C = cumsum(tag); ordinal = C*tag - 1 (chunk-rebased), as i16
        C = pool.tile([P, SG], F32, tag="C")
        nc.vector.tensor_tensor_scan(C[:], tagF[:], tagF[:], 0.0,
                                     Alu.add, Alu.bypass)
        nc.vector.tensor_tensor(C[:], C[:], tagF[:], Alu.mult)
        idx16 = pool.tile([P, SG], I16, tag="idx16")
        for c in range(n_pos_chunks):
            sl = slice(c * pos_pos, (c + 1) * pos_pos)
            nc.vector.tensor_scalar(out=idx16[:, sl], in0=C[:, sl],
                                    scalar1=float(-1 - c * pos_ord),
                                    scalar2=None, op0=Alu.add)
        # posTab[ordinal] = chunk-local position
        posTab = pool.tile([P, ORD], I16, tag="posTab")
        for c in range(n_pos_chunks):
            nc.gpsimd.local_scatter(
                posTab[:, c * pos_ord:(c + 1) * pos_ord],
                iotaPos[:, c * pos_pos:(c + 1) * pos_pos],
                idx16[:, c * pos_pos:(c + 1) * pos_pos],
                channels=P, num_elems=pos_ord, num_idxs=pos_pos)
        # scatter d16/s16 to breakpoint positions
        dS = pool.tile([P, SG], F16, tag="dS")
        sS = pool.tile([P, SG], F16, tag="sS")
        for c in range(n_ds_chunks):
            osl = slice(c * ds_ord, (c + 1) * ds_ord)
            psl = slice(c * ds_pos, (c + 1) * ds_pos)
            nc.gpsimd.local_scatter(dS[:, psl], d16T[:, osl], posTab[:, osl],
                                    channels=P, num_elems=ds_pos,
                                    num_idxs=ds_ord)
            nc.gpsimd.local_scatter(sS[:, psl], s16T[:, osl], posTab[:, osl],
                                    channels=P, num_elems=ds_pos,
                                    num_idxs=ds_ord)
        # ffills (in place)
        nc.vector.tensor_tensor_scan(dS[:], omt[:], dS[:], 0.0,
                                     Alu.mult, Alu.add)
        nc.vector.tensor_tensor_scan(sS[:], omt[:], sS[:], 0.0,
                                     Alu.mult, Alu.add)
        # cdf0 ffill: cdfAt = v*tag (denorm is -0.0-ish -> ~0)
        cdfAt = pool.tile([P, SG], F32, tag="cdfAt")
        nc.vector.tensor_tensor(_r3(cdfAt[:], OUT), Mv, _r3(tagF[:], OUT),
                                Alu.mult)
        nc.vector.tensor_tensor_scan(cdfAt[:], omt[:], cdfAt[:], 0.0,
                                     Alu.mult, Alu.add)
        # out = d0 + max(v - cdf0, 0) * s0   (s0 reuses omt's buffer,
        # d0 reuses M's buffer -- both dead by now)
        nc.vector.tensor_tensor(_r3(tagF[:], OUT), Mv, _r3(cdfAt[:], OUT),
                                Alu.subtract)
        s0 = omt
        nc.vector.tensor_copy(s0[:], sS[:])
        nc.vector.scalar_tensor_tensor(s0[:], tagF[:], 0.0, s0[:],
                                       Alu.max, Alu.mult)
        nc.vector.tensor_copy(M[:, 0:SG], dS[:])
        nc.vector.tensor_tensor(s0[:], s0[:], M[:, 0:SG], Alu.add)
        nc.sync.dma_start(
            out_ap[rays0:rays1, :].rearrange("(p k) c -> p (k c)", p=P),
            s0[:])


def build_body(tc, ctx, nc, dists_ap, weights_ap, u_ap, out_ap, n_tiles,
               repeat=1):
    dram = ctx.enter_context(
        tc.tile_pool(name="scratch", bufs=1, space="DRAM"))
    n_rays = n_tiles * P
    cN_t = dram.tile([n_rays, NW], F32)
    d16_t = dram.tile([n_rays, SC], F16)
    s16_t = dram.tile([n_rays, SC], F16)
    usort_t = dram.tile([n_rays, SF], F32)
    merged_t = dram.tile([n_rays, W], F32)

    for _ in range(repeat):
        with ExitStack() as pctx:
            emit_pass0(nc, tc, pctx, dists_ap, weights_ap, cN_t[:], d16_t[:],
                       s16_t[:], n_tiles)
        with ExitStack() as pctx:
            emit_pass1(nc, tc, pctx, u_ap, usort_t[:], n_tiles)
        with ExitStack() as pctx:
            emit_pass2(nc, tc, pctx, usort_t[:], cN_t[:], merged_t[:], n_tiles)
        with ExitStack() as pctx:
            emit_pass3(nc, tc, pctx, merged_t[:], d16_t[:], s16_t[:], out_ap,
                       n_tiles)


def build_kernel(n_rays, repeat=1):
    assert n_rays % (P * G3_MAX) == 0
    nc = bacc.Bacc("TRN2", target_bir_lowering=False, debug=False)
    dists = nc.dram_tensor("dists", [n_rays, SC], F32,
                           kind="ExternalInput").ap()
    weights = nc.dram_tensor("weights", [n_rays, NW], F32,
                             kind="ExternalInput").ap()
    u = nc.dram_tensor("u", [n_rays, SF], F32, kind="ExternalInput").ap()
    out = nc.dram_tensor("out", [n_rays, OUT], F32,
                         kind="ExternalOutput").ap()
    with tile.TileContext(nc) as tc:
        with ExitStack() as ctx:
            build_body(tc, ctx, nc, dists, weights, u, out, n_rays // P,
                       repeat=repeat)
    nc.compile()
    return nc


N_CORES = 8
B_FULL = 262144
R_CORE = B_FULL // N_CORES   # 32768 rays per core

_NC_CACHE = {}


def _get_nc(n_rays, repeat=1):
    key = (n_rays, repeat)
    if key not in _NC_CACHE:
        _NC_CACHE[key] = build_kernel(n_rays, repeat)
    return _NC_CACHE[key]


def kernel(dists, weights, uniform_rands, samples_fine):
    from concourse.bass_utils import run_bass_kernel_spmd
    dists = np.ascontiguousarray(np.asarray(dists, dtype=np.float32))
    weights = np.ascontiguousarray(np.asarray(weights, dtype=np.float32))
    u = np.ascontiguousarray(np.asarray(uniform_rands, dtype=np.float32))
    assert int(samples_fine) == SF
    B = dists.shape[0]
    assert B == B_FULL and dists.shape[1] == SC and weights.shape[1] == NW \
        and u.shape[1] == SF

    nc = _get_nc(R_CORE)
    in_maps = []
    for c in range(N_CORES):
        r0, r1 = c * R_CORE, (c + 1) * R_CORE
        in_maps.append({"dists": dists[r0:r1], "weights": weights[r0:r1],
                        "u": u[r0:r1]})
    res = run_bass_kernel_spmd(nc, in_maps, list(range(N_CORES)))
    return np.concatenate([res.results[c]["out"] for c in range(N_CORES)],
                          axis=0)
